# revision 11
# baseline (speedup 1.0000x reference)
"""DeepseekV2 MLA attention on 8 Trainium2 NeuronCores (Bass/Tile).

Tensor-parallel over heads (2 heads/core): w_q_b / w_kv_b output dims and
w_o input dim sharded across cores; q_a / kv_a projections replicated.
Per-core partial outputs are summed on the host (row-parallel unshard).

Self-contained: hardcodes all shapes from the problem spec.
"""

import numpy as np

import concourse.bass as bass
import concourse.bacc as bacc
import concourse.mybir as mybir
import concourse.tile as tile
from concourse import bass_utils

# Problem dims
T = 2048
HID = 2048
H = 16
DN = 128      # qk_nope_head_dim
DR = 64       # qk_rope_head_dim
DV = 128      # v_head_dim
DQK = DN + DR
QLR = 1536    # q_lora_rank
KVLR = 512    # kv_lora_rank
THETA = 10000.0
EPS = 1e-6
SCALE = DQK ** -0.5

NCORES = 8
HPC = H // NCORES            # heads per core = 2
LATR = KVLR + DR             # latent rows = 576

F32 = mybir.dt.float32
F32R = mybir.dt.float32r

KT = HID // 128              # 16 contraction chunks for phase A
QMT = QLR // 128             # 12 q_a row tiles
KVMT = KVLR // 128           # 4 latent (normed) row tiles
NB = T // 512                # 4 token blocks of 512
TBT = T // 128               # 16 token tiles of 128





def build_bass():
    nc = bacc.Bacc(
        "TRN2",
        target_bir_lowering=False,
        debug=False,
        enable_asserts=False,
        num_devices=NCORES,
    )

    hs_t = nc.dram_tensor("hs_t", [HID, T], F32R, kind="ExternalInput").ap()
    wqa = nc.dram_tensor("wqa", [HID, QLR], F32R, kind="ExternalInput").ap()
    wkva = nc.dram_tensor("wkva", [HID, LATR], F32R, kind="ExternalInput").ap()
    wqb = nc.dram_tensor("wqb", [QLR, HPC * DQK], F32R, kind="ExternalInput").ap()
    wkvbk = nc.dram_tensor("wkvbk", [KVLR, HPC * DN], F32R, kind="ExternalInput").ap()
    wkvbv = nc.dram_tensor("wkvbv", [KVLR, HPC * DV], F32R, kind="ExternalInput").ap()
    wo = nc.dram_tensor("wo", [HPC * DV, HID], F32R, kind="ExternalInput").ap()
    cosf = nc.dram_tensor("cosf", [DR, T], F32R, kind="ExternalInput").ap()
    sinf = nc.dram_tensor("sinf", [DR, T], F32R, kind="ExternalInput").ap()
    perm64 = nc.dram_tensor("perm64", [DR, DR], F32R, kind="ExternalInput").ap()
    maskd = nc.dram_tensor("maskd", [128, 4 * 512], F32R, kind="ExternalInput").ap()
    ones = nc.dram_tensor("ones", [128, 128], F32R, kind="ExternalInput").ap()
    out = nc.dram_tensor("out", [T, HID], F32, kind="ExternalOutput").ap()

    with tile.TileContext(nc) as tc:
        _kernel_body(nc, tc, hs_t, wqa, wkva, wqb, wkvbk, wkvbv, wo,
                     cosf, sinf, perm64, maskd, ones, out)

    nc.compile()
    return nc


def _kernel_body(nc, tc, hs_t, wqa, wkva, wqb, wkvbk, wkvbv, wo,
                 cosf, sinf, perm64, maskd, ones, out):
    from contextlib import ExitStack

    ctx = ExitStack()
    with ctx:
        dram = ctx.enter_context(tc.tile_pool(name="dram", bufs=1, space="DRAM"))
        qa_sc = dram.tile([QLR, T], F32R)
        lat_sc = dram.tile([LATR, T], F32R)

        persist = ctx.enter_context(tc.tile_pool(name="persist", bufs=1))

        ones128 = persist.tile([128, 128], F32R, tag="ones128")
        nc.sync.dma_start(out=ones128, in_=ones)
        ones_col = ones128[:, 0:1]
        ones_row = ones128[0:1, :]

        s_q = persist.tile([1, T], F32R, tag="s_q")
        s_kv = persist.tile([1, T], F32R, tag="s_kv")

        # ---------------- Phase A: q_a^T and latent^T (replicated) -------------
        with tc.tile_pool(name="pa", bufs=1) as pa, \
             tc.tile_pool(name="psa", bufs=1, space="PSUM") as psa:
            hst = []
            for k in range(KT):
                h = pa.tile([128, T], F32R, tag=f"hs{k}")
                nc.sync.dma_start(out=h, in_=hs_t[k * 128:(k + 1) * 128, :])
                hst.append(h)

            zq = psa.tile([1, T], F32, tag="z")

            def a_mtile(w_dram, m_cols, mrows, do_z, z_tile, z_start, z_stop, out_dram_rows, out_dram):
                # w strip [HID, mrows] -> [128, KT, mrows]
                wstrip = pa.tile([128, KT, mrows], F32R, tag="wstrip", bufs=2)
                nc.scalar.dma_start(
                    out=wstrip,
                    in_=w_dram[:, m_cols[0]:m_cols[1]].rearrange(
                        "(kc p) m -> p kc m", p=128),
                )
                stage = pa.tile([mrows, T], F32R, tag="stage", bufs=2)
                for half in range(2):
                    pq = psa.tile([mrows, 1024], F32, tag="pq", bufs=2)
                    for k in range(KT):
                        for nb in range(2):
                            nc.tensor.matmul(
                                pq[:, nb * 512:(nb + 1) * 512],
                                lhsT=wstrip[:, k, :],
                                rhs=hst[k][:, (2 * half + nb) * 512:
                                              (2 * half + nb + 1) * 512],
                                start=(k == 0),
                                stop=(k == KT - 1),
                            )
                    nc.vector.tensor_copy(
                        stage[:, half * 1024:(half + 1) * 1024], pq)
                    if do_z:
                        for nb in range(2):
                            gnb = 2 * half + nb
                            sq = pa.tile([mrows, 512], F32R, tag="sq", bufs=2)
                            nc.scalar.square(
                                sq, stage[:, gnb * 512:(gnb + 1) * 512])
                            nc.tensor.matmul(
                                z_tile[0:1, gnb * 512:(gnb + 1) * 512],
                                lhsT=ones_col[0:mrows, :],
                                rhs=sq,
                                start=z_start,
                                stop=z_stop,
                            )
                nc.sync.dma_start(
                    out=out_dram[out_dram_rows[0]:out_dram_rows[1], :], in_=stage)

            def rsqrt_row(dst, z_psum, n):
                # dst = 1/sqrt(z/n + eps) via DVE reciprocal + ACT sqrt
                tmp = pa.tile([1, T], F32, tag="rsq_tmp", bufs=1)
                nc.scalar.activation(tmp, z_psum,
                                     mybir.ActivationFunctionType.Copy,
                                     bias=EPS, scale=1.0 / n)
                nc.vector.reciprocal(tmp, tmp)
                nc.scalar.activation(dst, tmp,
                                     mybir.ActivationFunctionType.Sqrt)

            for m in range(QMT):
                a_mtile(wqa, (m * 128, (m + 1) * 128), 128, True, zq,
                        m == 0, m == QMT - 1, (m * 128, (m + 1) * 128), qa_sc)
            rsqrt_row(s_q, zq, QLR)

            zkv = psa.tile([1, T], F32, tag="z")
            for m in range(KVMT):
                a_mtile(wkva, (m * 128, (m + 1) * 128), 128, True, zkv,
                        m == 0, m == KVMT - 1, (m * 128, (m + 1) * 128), lat_sc)
            rsqrt_row(s_kv, zkv, KVLR)

            # k_pe rows (no norm)
            a_mtile(wkva, (KVLR, LATR), DR, False, None, False, False,
                    (KVLR, LATR), lat_sc)

        # ---------------- Phase B: per-head q/k/v + rope ------------------------
        qn = [[None] * NB for _ in range(HPC)]     # [128, 512] nope q, f-major
        qpe = [[None] * NB for _ in range(HPC)]    # [64, 512] roped q pe
        kn = [[None] * NB for _ in range(HPC)]     # [128, 512] k nope, f-major
        kpe = [None] * NB                          # [64, 512] roped k pe (shared)
        vt = [None] * TBT                          # [128, 256] v token-major, 2 heads

        bcp = ctx.enter_context(tc.tile_pool(name="bcp", bufs=1))

        with tc.tile_pool(name="pb", bufs=1) as pb, \
             tc.tile_pool(name="psb", bufs=1, space="PSUM") as psb:
            # resident weights
            wqb_t = pb.tile([128, QMT, HPC * DQK], F32R, tag="wqb")
            nc.sync.dma_start(
                out=wqb_t, in_=wqb.rearrange("(kc p) m -> p kc m", p=128))
            wkvbk_t = pb.tile([128, KVMT, HPC * DN], F32R, tag="wkvbk")
            nc.sync.dma_start(
                out=wkvbk_t, in_=wkvbk.rearrange("(kc p) m -> p kc m", p=128))
            wkvbv_t = pb.tile([128, KVMT, HPC * DV], F32R, tag="wkvbv")
            nc.sync.dma_start(
                out=wkvbv_t, in_=wkvbv.rearrange("(kc p) m -> p kc m", p=128))
            cosf_t = pb.tile([DR, T], F32R, tag="cosf")
            nc.sync.dma_start(out=cosf_t, in_=cosf)
            sinf_t = pb.tile([DR, T], F32R, tag="sinf")
            nc.sync.dma_start(out=sinf_t, in_=sinf)
            perm_t = pb.tile([DR, DR], F32R, tag="perm")
            nc.sync.dma_start(out=perm_t, in_=perm64)

            def rope(dst, raw, blk):
                """raw [64, 512] (evens;odds) -> roped dst [64, 512]."""
                sl = slice(blk * 512, (blk + 1) * 512)
                sw_ps = psb.tile([DR, 512], F32, tag="swp", bufs=2)
                nc.tensor.matmul(sw_ps, lhsT=perm_t, rhs=raw,
                                 start=True, stop=True)
                t1 = pb.tile([DR, 512], F32R, tag="ropet1", bufs=2)
                nc.vector.tensor_tensor(t1, raw, cosf_t[:, sl],
                                        op=mybir.AluOpType.mult)
                t2 = pb.tile([DR, 512], F32R, tag="ropet2", bufs=2)
                nc.vector.tensor_tensor(t2, sw_ps, sinf_t[:, sl],
                                        op=mybir.AluOpType.mult)
                nc.vector.tensor_tensor(dst, t1, t2, op=mybir.AluOpType.add)

            for j in range(NB):
                jsl = slice(j * 512, (j + 1) * 512)
                # broadcast rms scales across partitions via K=1 outer product
                bq_ps = psb.tile([128, 512], F32, tag="bc", bufs=2)
                nc.tensor.matmul(bq_ps, lhsT=ones_row, rhs=s_q[:, jsl],
                                 start=True, stop=True)
                sq_bc = pb.tile([128, 512], F32R, tag="sqbc", bufs=2)
                nc.scalar.copy(sq_bc, bq_ps)
                bkv_ps = psb.tile([128, 512], F32, tag="bc", bufs=2)
                nc.tensor.matmul(bkv_ps, lhsT=ones_row, rhs=s_kv[:, jsl],
                                 start=True, stop=True)
                skv_bc = pb.tile([128, 512], F32R, tag="skvbc", bufs=2)
                nc.scalar.copy(skv_bc, bkv_ps)

                # ---- q_b matmuls: 4 out tiles (h0n, h1n, h0pe, h1pe) ----
                accs = []
                for mt, rows in ((0, 128), (1, 128), (2, DR), (3, DR)):
                    accs.append(psb.tile([rows, 512], F32, tag="acc", bufs=4, name=f"accq{mt}"))
                col_of = (0, DN, 2 * DN, 2 * DN + DR)
                rows_of = (128, 128, DR, DR)
                for k in range(QMT):
                    qa_ch = pb.tile([128, 512], F32R, tag="qa", bufs=4)
                    eng = nc.gpsimd if k % 2 == 0 else nc.sync
                    eng.dma_start(
                        out=qa_ch, in_=qa_sc[k * 128:(k + 1) * 128, jsl])
                    for mt in range(4):
                        nc.tensor.matmul(
                            accs[mt],
                            lhsT=wqb_t[:, k, col_of[mt]:col_of[mt] + rows_of[mt]],
                            rhs=qa_ch,
                            start=(k == 0),
                            stop=(k == QMT - 1),
                        )
                for h in range(HPC):
                    qn[h][j] = bcp.tile([128, 512], F32R, tag=f"qn{h}_{j}", name=f"qn{h}_{j}")
                    nc.vector.tensor_tensor(qn[h][j], accs[h], sq_bc,
                                            op=mybir.AluOpType.mult)
                for h in range(HPC):
                    qpe_raw = pb.tile([DR, 512], F32R, tag="qperaw", bufs=2)
                    nc.vector.tensor_tensor(qpe_raw, accs[2 + h], sq_bc[0:DR, :],
                                            op=mybir.AluOpType.mult)
                    qpe[h][j] = bcp.tile([DR, 512], F32R, tag=f"qpe{h}_{j}", name=f"qpe{h}_{j}")
                    rope(qpe[h][j], qpe_raw, j)

                # ---- latent chunks -> normalized kv_a^T + roped k_pe ----
                kva_n = []
                for k in range(KVMT):
                    lat_ch = pb.tile([128, 512], F32R, tag="lat", bufs=2)
                    nc.gpsimd.dma_start(
                        out=lat_ch, in_=lat_sc[k * 128:(k + 1) * 128, jsl])
                    kvn = pb.tile([128, 512], F32R, tag="kvan", bufs=4)
                    nc.vector.tensor_tensor(kvn, lat_ch, skv_bc,
                                            op=mybir.AluOpType.mult)
                    kva_n.append(kvn)
                kpe_raw = pb.tile([DR, 512], F32R, tag="kperaw", bufs=2)
                nc.gpsimd.dma_start(out=kpe_raw, in_=lat_sc[KVLR:LATR, jsl])
                kpe[j] = bcp.tile([DR, 512], F32R, tag=f"kpe_{j}", name=f"kpe_{j}")
                rope(kpe[j], kpe_raw, j)

                # ---- k_nope ----
                for h in range(HPC):
                    acck = psb.tile([128, 512], F32, tag="acc", bufs=4)
                    for k in range(KVMT):
                        nc.tensor.matmul(
                            acck,
                            lhsT=wkvbk_t[:, k, h * DN:(h + 1) * DN],
                            rhs=kva_n[k],
                            start=(k == 0),
                            stop=(k == KVMT - 1),
                        )
                    kn[h][j] = bcp.tile([128, 512], F32R, tag=f"kn{h}_{j}", name=f"kn{h}_{j}")
                    nc.vector.tensor_copy(kn[h][j], acck)

                # ---- v (token-major, both heads packed) ----
                for tt in range(4):
                    tb = j * 4 + tt
                    accv = psb.tile([128, HPC * DV], F32, tag="acc", bufs=4)
                    for k in range(KVMT):
                        nc.tensor.matmul(
                            accv,
                            lhsT=kva_n[k][:, tt * 128:(tt + 1) * 128],
                            rhs=wkvbv_t[:, k, :],
                            start=(k == 0),
                            stop=(k == KVMT - 1),
                        )
                    vt[tb] = bcp.tile([128, HPC * DV], F32R, tag=f"v_{tb}", name=f"v_{tb}")
                    nc.vector.tensor_copy(vt[tb], accv)

        # ---------------- Attention + output projection -------------------------
        with tc.tile_pool(name="pc", bufs=1) as pc, \
             tc.tile_pool(name="psc", bufs=1, space="PSUM") as psc:
            maskd_t = pc.tile([128, 4 * 512], F32R, tag="maskd")
            nc.sync.dma_start(out=maskd_t, in_=maskd)
            wo_t = []
            for h in range(HPC):
                w = pc.tile([128, HID], F32R, tag=f"wo{h}")
                nc.sync.dma_start(out=w, in_=wo[h * DV:(h + 1) * DV, :])
                wo_t.append(w)

            attn_n = [[None] * NB for _ in range(HPC)]
            for qj in range(NB):
                nki = 4 * qj + 4
                for h in range(HPC):
                    attn_ps = psc.tile([128, 512], F32, tag="attn", bufs=2)
                    z_ps = psc.tile([1, 512], F32, tag="zr", bufs=1)
                    for ki in range(nki):
                        jb, sub = ki // 4, ki % 4
                        ksl = slice(sub * 128, (sub + 1) * 128)
                        s_ps = psc.tile([128, 512], F32, tag="s", bufs=3)
                        nc.tensor.matmul(s_ps, lhsT=kn[h][jb][:, ksl],
                                         rhs=qn[h][qj],
                                         start=True, stop=False)
                        nc.tensor.matmul(s_ps, lhsT=kpe[jb][:, ksl],
                                         rhs=qpe[h][qj],
                                         start=False, stop=True)
                        e = pc.tile([128, 512], F32R, tag="e", bufs=4)
                        nc.scalar.activation(e, s_ps,
                                             mybir.ActivationFunctionType.Exp)
                        if ki >= 4 * qj:  # diagonal block: causal mask
                            sub_d = ki - 4 * qj
                            nc.vector.tensor_tensor(
                                e, e, maskd_t[:, sub_d * 512:(sub_d + 1) * 512],
                                op=mybir.AluOpType.mult)
                        nc.tensor.matmul(z_ps, lhsT=ones_col, rhs=e,
                                         start=(ki == 0), stop=(ki == nki - 1))
                        nc.tensor.matmul(attn_ps,
                                         lhsT=vt[ki][:, h * DV:(h + 1) * DV],
                                         rhs=e,
                                         start=(ki == 0), stop=(ki == nki - 1))
                    rz = pc.tile([1, 512], F32R, tag="rz", bufs=2)
                    with nc.allow_low_precision(reason="fp32r softmax denom"):
                        nc.vector.reciprocal(rz, z_ps)
                    bc_ps = psc.tile([128, 512], F32, tag="s", bufs=3)
                    nc.tensor.matmul(bc_ps, lhsT=ones_row, rhs=rz,
                                     start=True, stop=True)
                    bc_sb = pc.tile([128, 512], F32R, tag="bcs", bufs=2)
                    nc.scalar.copy(bc_sb, bc_ps)
                    attn_n[h][qj] = bcp.tile([128, 512], F32R, tag=f"attn{h}_{qj}", name=f"attn{h}_{qj}")
                    nc.vector.tensor_tensor(attn_n[h][qj], attn_ps, bc_sb,
                                            op=mybir.AluOpType.mult)

                # output projection for this qj's 4 token tiles
                for tt in range(4):
                    tb = qj * 4 + tt
                    tsl = slice(tt * 128, (tt + 1) * 128)
                    o_row = pc.tile([128, HID], F32, tag="orow", bufs=2)
                    for hb in range(NB):
                        o_ps = psc.tile([128, 512], F32, tag="o", bufs=2)
                        for h in range(HPC):
                            nc.tensor.matmul(
                                o_ps,
                                lhsT=attn_n[h][qj][:, tsl],
                                rhs=wo_t[h][:, hb * 512:(hb + 1) * 512],
                                start=(h == 0),
                                stop=(h == HPC - 1),
                            )
                        nc.vector.tensor_copy(o_row[:, hb * 512:(hb + 1) * 512], o_ps)
                    nc.gpsimd.dma_start(
                        out=out[tb * 128:(tb + 1) * 128, :], in_=o_row)


# ------------------------------ host side ----------------------------------

_NC_CACHE = {}


def _get_nc():
    if "nc" not in _NC_CACHE:
        _NC_CACHE["nc"] = build_bass()
    return _NC_CACHE["nc"]


def make_in_maps(positions, hidden_states, w_q_a, q_a_ln_w, w_q_b, w_kv_a,
                 kv_a_ln_w, w_kv_b, w_o):
    positions = np.asarray(positions)
    hidden_states = np.asarray(hidden_states, dtype=np.float32)
    w_q_a = np.asarray(w_q_a, dtype=np.float32)
    q_a_ln_w = np.asarray(q_a_ln_w, dtype=np.float32)
    w_q_b = np.asarray(w_q_b, dtype=np.float32)
    w_kv_a = np.asarray(w_kv_a, dtype=np.float32)
    kv_a_ln_w = np.asarray(kv_a_ln_w, dtype=np.float32)
    w_kv_b = np.asarray(w_kv_b, dtype=np.float32)
    w_o = np.asarray(w_o, dtype=np.float32)

    hs_t = np.ascontiguousarray(hidden_states.T)

    # de-interleave order for rope dims: evens then odds
    order = np.concatenate([np.arange(0, DR, 2), np.arange(1, DR, 2)])

    wkva_p = w_kv_a.copy()
    wkva_p[:, KVLR:] = w_kv_a[:, KVLR:][:, order]
    wkva_p = np.ascontiguousarray(wkva_p)

    # rope tables (feature-major, de-interleaved: evens;odds)
    inv_freq = 1.0 / (THETA ** (np.arange(0, DR, 2, dtype=np.float64) / DR))
    ang = positions.astype(np.float64)[:, None] * inv_freq[None, :]  # [T, 32]
    cosT = np.cos(ang).T.astype(np.float32)                          # [32, T]
    sinT = np.sin(ang).T.astype(np.float32)
    cosf = np.ascontiguousarray(np.concatenate([cosT, cosT], axis=0))
    sinf = np.ascontiguousarray(np.concatenate([-sinT, sinT], axis=0))

    # block swap permutation (lhsT form; symmetric)
    perm = np.zeros((DR, DR), dtype=np.float32)
    for i in range(DR):
        perm[i, (i + DR // 2) % DR] = 1.0

    # diagonal causal mask patterns: keep if p + 128*sub <= f
    maskd = np.zeros((128, 4 * 512), dtype=np.float32)
    p = np.arange(128)[:, None]
    f = np.arange(512)[None, :]
    for sub in range(4):
        maskd[:, sub * 512:(sub + 1) * 512] = (p + 128 * sub <= f)
    maskd = np.ascontiguousarray(maskd)

    in_maps = []
    for c in range(NCORES):
        h0, h1 = HPC * c, HPC * c + 1
        wqb_c = np.concatenate([
            w_q_b[:, h0 * DQK:h0 * DQK + DN],
            w_q_b[:, h1 * DQK:h1 * DQK + DN],
            w_q_b[:, h0 * DQK + DN:(h0 + 1) * DQK][:, order],
            w_q_b[:, h1 * DQK + DN:(h1 + 1) * DQK][:, order],
        ], axis=1) * q_a_ln_w[:, None] * SCALE
        wkvbk_c = np.concatenate([
            w_kv_b[:, h0 * (DN + DV):h0 * (DN + DV) + DN],
            w_kv_b[:, h1 * (DN + DV):h1 * (DN + DV) + DN],
        ], axis=1) * kv_a_ln_w[:, None]
        wkvbv_c = np.concatenate([
            w_kv_b[:, h0 * (DN + DV) + DN:(h0 + 1) * (DN + DV)],
            w_kv_b[:, h1 * (DN + DV) + DN:(h1 + 1) * (DN + DV)],
        ], axis=1) * kv_a_ln_w[:, None]
        wo_c = np.concatenate([
            w_o[h0 * DV:(h0 + 1) * DV, :],
            w_o[h1 * DV:(h1 + 1) * DV, :],
        ], axis=0)
        in_maps.append({
            "hs_t": hs_t,
            "wqa": w_q_a,
            "wkva": wkva_p,
            "wqb": np.ascontiguousarray(wqb_c.astype(np.float32)),
            "wkvbk": np.ascontiguousarray(wkvbk_c.astype(np.float32)),
            "wkvbv": np.ascontiguousarray(wkvbv_c.astype(np.float32)),
            "wo": np.ascontiguousarray(wo_c.astype(np.float32)),
            "cosf": cosf,
            "sinf": sinf,
            "perm64": perm,
            "maskd": maskd,
            "ones": np.ones((128, 128), dtype=np.float32),
        })
    return in_maps


def kernel(positions, hidden_states, w_q_a, q_a_ln_w, w_q_b, w_kv_a,
           kv_a_ln_w, w_kv_b, w_o):
    nc = _get_nc()
    in_maps = make_in_maps(positions, hidden_states, w_q_a, q_a_ln_w, w_q_b,
                           w_kv_a, kv_a_ln_w, w_kv_b, w_o)
    res = bass_utils.run_bass_kernel_spmd(nc, in_maps, core_ids=list(range(NCORES)))
    acc = np.zeros((T, HID), dtype=np.float32)
    for c in range(NCORES):
        acc += res.results[c]["out"]
    return acc


# revision 15
# speedup vs baseline: 1.0735x; 1.0735x over previous
"""DeepseekV2 MLA attention on 8 Trainium2 NeuronCores (Bass/Tile).

Tensor-parallel over heads (2 heads/core): w_q_b / w_kv_b output dims and
w_o input dim sharded across cores; q_a / kv_a projections replicated.
Per-core partial outputs are summed on the host (row-parallel unshard).

Self-contained: hardcodes all shapes from the problem spec.
"""

import numpy as np

import concourse.bass as bass
import concourse.bacc as bacc
import concourse.mybir as mybir
import concourse.tile as tile
from concourse import bass_utils

# Problem dims
T = 2048
HID = 2048
H = 16
DN = 128      # qk_nope_head_dim
DR = 64       # qk_rope_head_dim
DV = 128      # v_head_dim
DQK = DN + DR
QLR = 1536    # q_lora_rank
KVLR = 512    # kv_lora_rank
THETA = 10000.0
EPS = 1e-6
SCALE = DQK ** -0.5

NCORES = 8
HPC = H // NCORES            # heads per core = 2
LATR = KVLR + DR             # latent rows = 576

F32 = mybir.dt.float32
F32R = mybir.dt.float32r

KT = HID // 128              # 16 contraction chunks for phase A
QMT = QLR // 128             # 12 q_a row tiles
KVMT = KVLR // 128           # 4 latent (normed) row tiles
NB = T // 512                # 4 token blocks of 512
TBT = T // 128               # 16 token tiles of 128
TSH = T // NCORES            # 256 tokens per core shard





def build_bass():
    nc = bacc.Bacc(
        "TRN2",
        target_bir_lowering=False,
        debug=False,
        enable_asserts=False,
        num_devices=NCORES,
    )

    hs_sh = nc.dram_tensor("hs_sh", [HID, TSH], F32R, kind="ExternalInput").ap()
    wqa = nc.dram_tensor("wqa", [HID, QLR], F32R, kind="ExternalInput").ap()
    wkva = nc.dram_tensor("wkva", [HID, LATR], F32R, kind="ExternalInput").ap()
    wqb = nc.dram_tensor("wqb", [QLR, HPC * DQK], F32R, kind="ExternalInput").ap()
    wkvbk = nc.dram_tensor("wkvbk", [KVLR, HPC * DN], F32R, kind="ExternalInput").ap()
    wkvbv = nc.dram_tensor("wkvbv", [KVLR, HPC * DV], F32R, kind="ExternalInput").ap()
    wo = nc.dram_tensor("wo", [HPC * DV, HID], F32R, kind="ExternalInput").ap()
    cosf = nc.dram_tensor("cosf", [DR, T], F32R, kind="ExternalInput").ap()
    sinf = nc.dram_tensor("sinf", [DR, T], F32R, kind="ExternalInput").ap()
    cosf_sh = nc.dram_tensor("cosf_sh", [DR, TSH], F32R, kind="ExternalInput").ap()
    sinf_sh = nc.dram_tensor("sinf_sh", [DR, TSH], F32R, kind="ExternalInput").ap()
    perm64 = nc.dram_tensor("perm64", [DR, DR], F32R, kind="ExternalInput").ap()
    maskd = nc.dram_tensor("maskd", [128, 4 * 512], F32R, kind="ExternalInput").ap()
    ones = nc.dram_tensor("ones", [128, 128], F32R, kind="ExternalInput").ap()
    out = nc.dram_tensor("out", [T, HID], F32, kind="ExternalOutput").ap()

    with tile.TileContext(nc) as tc:
        _kernel_body(nc, tc, hs_sh, wqa, wkva, wqb, wkvbk, wkvbv, wo,
                     cosf, sinf, cosf_sh, sinf_sh, perm64, maskd, ones, out)

    nc.compile()
    return nc


def _kernel_body(nc, tc, hs_sh, wqa, wkva, wqb, wkvbk, wkvbv, wo,
                 cosf, sinf, cosf_sh, sinf_sh, perm64, maskd, ones, out):
    from contextlib import ExitStack

    ctx = ExitStack()
    with ctx:
        dram = ctx.enter_context(tc.tile_pool(name="dram", bufs=1, space="DRAM"))
        contrib_q = dram.tile([QLR, TSH], F32R)
        contrib_kv = dram.tile([LATR, TSH], F32R)
        gath_q = dram.tile([NCORES * QLR, TSH], F32R, addr_space="Shared")
        gath_kv = dram.tile([NCORES * LATR, TSH], F32R, addr_space="Shared")

        persist = ctx.enter_context(tc.tile_pool(name="persist", bufs=1))

        ones128 = persist.tile([128, 128], F32R, tag="ones128")
        nc.sync.dma_start(out=ones128, in_=ones)
        ones_col = ones128[:, 0:1]
        ones_row = ones128[0:1, :]
        perm_t0 = persist.tile([DR, DR], F32R, tag="perm0")
        nc.sync.dma_start(out=perm_t0, in_=perm64)
        cosf_sh_t = persist.tile([DR, TSH], F32R, tag="cosfsh")
        nc.sync.dma_start(out=cosf_sh_t, in_=cosf_sh)
        sinf_sh_t = persist.tile([DR, TSH], F32R, tag="sinfsh")
        nc.sync.dma_start(out=sinf_sh_t, in_=sinf_sh)


        # ------- Phase A: token-sharded q_a^T / latent^T, norm + rope local ----
        with tc.tile_pool(name="pa", bufs=1) as pa, \
             tc.tile_pool(name="psa", bufs=1, space="PSUM") as psa:
            hst = []
            for k in range(KT):
                h = pa.tile([128, TSH], F32R, tag=f"hs{k}")
                nc.sync.dma_start(out=h, in_=hs_sh[k * 128:(k + 1) * 128, :])
                hst.append(h)

            def a_mtile(w_dram, m_cols, mrows, z_tile, z_start, z_stop, stg_tag):
                wstrip = pa.tile([128, KT, mrows], F32R, tag="wstrip", bufs=2)
                nc.scalar.dma_start(
                    out=wstrip,
                    in_=w_dram[:, m_cols[0]:m_cols[1]].rearrange(
                        "(kc p) m -> p kc m", p=128),
                )
                pq = psa.tile([mrows, TSH], F32, tag="pq", bufs=3)
                for k in range(KT):
                    nc.tensor.matmul(
                        pq, lhsT=wstrip[:, k, :], rhs=hst[k],
                        start=(k == 0), stop=(k == KT - 1))
                stage = pa.tile([mrows, TSH], F32R, tag=stg_tag, name=stg_tag)
                nc.vector.tensor_copy(stage, pq)
                if z_tile is not None:
                    sq = pa.tile([mrows, TSH], F32R, tag="sq", bufs=2)
                    nc.scalar.square(sq, stage)
                    nc.tensor.matmul(z_tile, lhsT=ones_col[0:mrows, :], rhs=sq,
                                     start=z_start, stop=z_stop)
                return stage

            def rsqrt_bc(z_psum, n, tag):
                # [128, TSH] broadcast of 1/sqrt(z/n + eps)
                tmp = pa.tile([1, TSH], F32, tag="rsq_tmp", bufs=2)
                nc.scalar.activation(tmp, z_psum,
                                     mybir.ActivationFunctionType.Copy,
                                     bias=EPS, scale=1.0 / n)
                nc.vector.reciprocal(tmp, tmp)
                srow = pa.tile([1, TSH], F32R, tag=tag + "r", name=tag + "r")
                nc.scalar.activation(srow, tmp,
                                     mybir.ActivationFunctionType.Sqrt)
                b_ps = psa.tile([128, TSH], F32, tag="bc", bufs=2)
                nc.tensor.matmul(b_ps, lhsT=ones_row, rhs=srow,
                                 start=True, stop=True)
                bc = pa.tile([128, TSH], F32R, tag=tag, name=tag)
                nc.scalar.copy(bc, b_ps)
                return bc

            zq = psa.tile([1, TSH], F32, tag="z")
            q_stages = []
            for m in range(QMT):
                q_stages.append(a_mtile(wqa, (m * 128, (m + 1) * 128), 128,
                                        zq, m == 0, m == QMT - 1, f"stq{m}"))
            sq_bc = rsqrt_bc(zq, QLR, "sqbc")

            zkv = psa.tile([1, TSH], F32, tag="z")
            kv_stages = []
            for m in range(KVMT):
                kv_stages.append(a_mtile(wkva, (m * 128, (m + 1) * 128), 128,
                                         zkv, m == 0, m == KVMT - 1, f"stkv{m}"))
            skv_bc = rsqrt_bc(zkv, KVLR, "skvbc")

            kpe_stage = a_mtile(wkva, (KVLR, LATR), DR, None, False, False,
                                "stkpe")

            # normalize + write contributions
            for m in range(QMT):
                qs = pa.tile([128, TSH], F32R, tag="qnorm", bufs=3,
                             name=f"qnorm{m}")
                nc.vector.tensor_tensor(qs, q_stages[m], sq_bc,
                                        op=mybir.AluOpType.mult)
                nc.sync.dma_start(
                    out=contrib_q[m * 128:(m + 1) * 128, :], in_=qs)
            for m in range(KVMT):
                ks = pa.tile([128, TSH], F32R, tag="kvnorm", bufs=2,
                             name=f"kvnorm{m}")
                nc.vector.tensor_tensor(ks, kv_stages[m], skv_bc,
                                        op=mybir.AluOpType.mult)
                nc.sync.dma_start(
                    out=contrib_kv[m * 128:(m + 1) * 128, :], in_=ks)
            # rope k_pe locally
            sw_ps = psa.tile([DR, TSH], F32, tag="bc", bufs=2)
            nc.tensor.matmul(sw_ps, lhsT=perm_t0, rhs=kpe_stage,
                             start=True, stop=True)
            rt1 = pa.tile([DR, TSH], F32R, tag="rt1")
            nc.vector.tensor_tensor(rt1, kpe_stage, cosf_sh_t,
                                    op=mybir.AluOpType.mult)
            rt2 = pa.tile([DR, TSH], F32R, tag="rt2")
            nc.vector.tensor_tensor(rt2, sw_ps, sinf_sh_t,
                                    op=mybir.AluOpType.mult)
            kpe_roped = pa.tile([DR, TSH], F32R, tag="kper")
            nc.vector.tensor_tensor(kpe_roped, rt1, rt2,
                                    op=mybir.AluOpType.add)
            nc.sync.dma_start(out=contrib_kv[KVLR:LATR, :], in_=kpe_roped)

            # all-gather (kv first so B's kv-side work unblocks early)
            nc.gpsimd.collective_compute(
                "AllGather", mybir.AluOpType.bypass,
                replica_groups=[list(range(NCORES))],
                ins=[contrib_kv], outs=[gath_kv])
            nc.gpsimd.collective_compute(
                "AllGather", mybir.AluOpType.bypass,
                replica_groups=[list(range(NCORES))],
                ins=[contrib_q], outs=[gath_q])

        # ---------------- Phase B: per-head q/k/v + rope ------------------------
        qn = [[None] * NB for _ in range(HPC)]     # [128, 512] nope q, f-major
        qpe = [[None] * NB for _ in range(HPC)]    # [64, 512] roped q pe
        kn = [[None] * NB for _ in range(HPC)]     # [128, 512] k nope, f-major
        kpe = [None] * NB                          # [64, 512] roped k pe (shared)
        vt = [None] * TBT                          # [128, 256] v token-major, 2 heads

        bcp = ctx.enter_context(tc.tile_pool(name="bcp", bufs=1))

        with tc.tile_pool(name="pb", bufs=1) as pb, \
             tc.tile_pool(name="psb", bufs=1, space="PSUM") as psb:
            # resident weights
            wqb_t = pb.tile([128, QMT, HPC * DQK], F32R, tag="wqb")
            nc.sync.dma_start(
                out=wqb_t, in_=wqb.rearrange("(kc p) m -> p kc m", p=128))
            wkvbk_t = pb.tile([128, KVMT, HPC * DN], F32R, tag="wkvbk")
            nc.sync.dma_start(
                out=wkvbk_t, in_=wkvbk.rearrange("(kc p) m -> p kc m", p=128))
            wkvbv_t = pb.tile([128, KVMT, HPC * DV], F32R, tag="wkvbv")
            nc.sync.dma_start(
                out=wkvbv_t, in_=wkvbv.rearrange("(kc p) m -> p kc m", p=128))
            cosf_t = pb.tile([DR, T], F32R, tag="cosf")
            nc.sync.dma_start(out=cosf_t, in_=cosf)
            sinf_t = pb.tile([DR, T], F32R, tag="sinf")
            nc.sync.dma_start(out=sinf_t, in_=sinf)
            perm_t = pb.tile([DR, DR], F32R, tag="perm")
            nc.sync.dma_start(out=perm_t, in_=perm64)

            def rope(dst, raw, blk):
                """raw [64, 512] (evens;odds) -> roped dst [64, 512]."""
                sl = slice(blk * 512, (blk + 1) * 512)
                sw_ps = psb.tile([DR, 512], F32, tag="swp", bufs=2)
                nc.tensor.matmul(sw_ps, lhsT=perm_t, rhs=raw,
                                 start=True, stop=True)
                t1 = pb.tile([DR, 512], F32R, tag="ropet1", bufs=2)
                nc.vector.tensor_tensor(t1, raw, cosf_t[:, sl],
                                        op=mybir.AluOpType.mult)
                t2 = pb.tile([DR, 512], F32R, tag="ropet2", bufs=2)
                nc.vector.tensor_tensor(t2, sw_ps, sinf_t[:, sl],
                                        op=mybir.AluOpType.mult)
                nc.vector.tensor_tensor(dst, t1, t2, op=mybir.AluOpType.add)

            for j in range(NB):
                jsl = slice(j * 512, (j + 1) * 512)
                # ---- q_b matmuls: 4 out tiles (h0n, h1n, h0pe, h1pe) ----
                accs = []
                for mt, rows in ((0, 128), (1, 128), (2, DR), (3, DR)):
                    accs.append(psb.tile([rows, 512], F32, tag="acc", bufs=4, name=f"accq{mt}"))
                col_of = (0, DN, 2 * DN, 2 * DN + DR)
                rows_of = (128, 128, DR, DR)
                for k in range(QMT):
                    qa_ch = pb.tile([128, 512], F32R, tag="qa", bufs=4)
                    eng = nc.scalar if k % 2 == 0 else nc.sync
                    for half in range(2):
                        base = (2 * j + half) * QLR + k * 128
                        eng.dma_start(
                            out=qa_ch[:, half * TSH:(half + 1) * TSH],
                            in_=gath_q[base:base + 128, :])
                    for mt in range(4):
                        nc.tensor.matmul(
                            accs[mt],
                            lhsT=wqb_t[:, k, col_of[mt]:col_of[mt] + rows_of[mt]],
                            rhs=qa_ch,
                            start=(k == 0),
                            stop=(k == QMT - 1),
                        )
                for h in range(HPC):
                    qn[h][j] = bcp.tile([128, 512], F32R, tag=f"qn{h}_{j}", name=f"qn{h}_{j}")
                    nc.vector.tensor_copy(qn[h][j], accs[h])
                for h in range(HPC):
                    qpe_raw = pb.tile([DR, 512], F32R, tag="qperaw", bufs=2)
                    nc.vector.tensor_copy(qpe_raw, accs[2 + h])
                    qpe[h][j] = bcp.tile([DR, 512], F32R, tag=f"qpe{h}_{j}", name=f"qpe{h}_{j}")
                    rope(qpe[h][j], qpe_raw, j)

                # ---- latent chunks (pre-normalized) + roped k_pe from gather ----
                kva_n = []
                for k in range(KVMT):
                    kvn = pb.tile([128, 512], F32R, tag="kvan", bufs=4)
                    for half in range(2):
                        base = (2 * j + half) * LATR + k * 128
                        nc.sync.dma_start(
                            out=kvn[:, half * TSH:(half + 1) * TSH],
                            in_=gath_kv[base:base + 128, :])
                    kva_n.append(kvn)
                kpe[j] = bcp.tile([DR, 512], F32R, tag=f"kpe_{j}", name=f"kpe_{j}")
                for half in range(2):
                    base = (2 * j + half) * LATR + KVLR
                    nc.sync.dma_start(
                        out=kpe[j][:, half * TSH:(half + 1) * TSH],
                        in_=gath_kv[base:base + DR, :])

                # ---- k_nope ----
                for h in range(HPC):
                    acck = psb.tile([128, 512], F32, tag="acc", bufs=4)
                    for k in range(KVMT):
                        nc.tensor.matmul(
                            acck,
                            lhsT=wkvbk_t[:, k, h * DN:(h + 1) * DN],
                            rhs=kva_n[k],
                            start=(k == 0),
                            stop=(k == KVMT - 1),
                        )
                    kn[h][j] = bcp.tile([128, 512], F32R, tag=f"kn{h}_{j}", name=f"kn{h}_{j}")
                    nc.vector.tensor_copy(kn[h][j], acck)

                # ---- v (token-major, both heads packed) ----
                for tt in range(4):
                    tb = j * 4 + tt
                    accv = psb.tile([128, HPC * DV], F32, tag="acc", bufs=4)
                    for k in range(KVMT):
                        nc.tensor.matmul(
                            accv,
                            lhsT=kva_n[k][:, tt * 128:(tt + 1) * 128],
                            rhs=wkvbv_t[:, k, :],
                            start=(k == 0),
                            stop=(k == KVMT - 1),
                        )
                    vt[tb] = bcp.tile([128, HPC * DV], F32R, tag=f"v_{tb}", name=f"v_{tb}")
                    nc.vector.tensor_copy(vt[tb], accv)

        # ---------------- Attention + output projection -------------------------
        with tc.tile_pool(name="pc", bufs=1) as pc, \
             tc.tile_pool(name="psc", bufs=1, space="PSUM") as psc:
            maskd_t = pc.tile([128, 4 * 512], F32R, tag="maskd")
            nc.sync.dma_start(out=maskd_t, in_=maskd)
            wo_t = []
            for h in range(HPC):
                w = pc.tile([128, HID], F32R, tag=f"wo{h}")
                nc.sync.dma_start(out=w, in_=wo[h * DV:(h + 1) * DV, :])
                wo_t.append(w)

            attn_n = [[None] * NB for _ in range(HPC)]
            for qj in range(NB):
                nki = 4 * qj + 4
                for h in range(HPC):
                    attn_ps = psc.tile([128, 512], F32, tag="attn", bufs=2)
                    z_ps = psc.tile([1, 512], F32, tag="zr", bufs=1)
                    for ki in range(nki):
                        jb, sub = ki // 4, ki % 4
                        ksl = slice(sub * 128, (sub + 1) * 128)
                        s_ps = psc.tile([128, 512], F32, tag="s", bufs=3)
                        nc.tensor.matmul(s_ps, lhsT=kn[h][jb][:, ksl],
                                         rhs=qn[h][qj],
                                         start=True, stop=False)
                        nc.tensor.matmul(s_ps, lhsT=kpe[jb][:, ksl],
                                         rhs=qpe[h][qj],
                                         start=False, stop=True)
                        e = pc.tile([128, 512], F32R, tag="e", bufs=4)
                        nc.scalar.activation(e, s_ps,
                                             mybir.ActivationFunctionType.Exp)
                        if ki >= 4 * qj:  # diagonal block: causal mask
                            sub_d = ki - 4 * qj
                            nc.vector.tensor_tensor(
                                e, e, maskd_t[:, sub_d * 512:(sub_d + 1) * 512],
                                op=mybir.AluOpType.mult)
                        nc.tensor.matmul(z_ps, lhsT=ones_col, rhs=e,
                                         start=(ki == 0), stop=(ki == nki - 1))
                        nc.tensor.matmul(attn_ps,
                                         lhsT=vt[ki][:, h * DV:(h + 1) * DV],
                                         rhs=e,
                                         start=(ki == 0), stop=(ki == nki - 1))
                    rz = pc.tile([1, 512], F32R, tag="rz", bufs=2)
                    with nc.allow_low_precision(reason="fp32r softmax denom"):
                        nc.vector.reciprocal(rz, z_ps)
                    bc_ps = psc.tile([128, 512], F32, tag="s", bufs=3)
                    nc.tensor.matmul(bc_ps, lhsT=ones_row, rhs=rz,
                                     start=True, stop=True)
                    bc_sb = pc.tile([128, 512], F32R, tag="bcs", bufs=2)
                    nc.scalar.copy(bc_sb, bc_ps)
                    attn_n[h][qj] = bcp.tile([128, 512], F32R, tag=f"attn{h}_{qj}", name=f"attn{h}_{qj}")
                    nc.vector.tensor_tensor(attn_n[h][qj], attn_ps, bc_sb,
                                            op=mybir.AluOpType.mult)

                # output projection for this qj's 4 token tiles
                for tt in range(4):
                    tb = qj * 4 + tt
                    tsl = slice(tt * 128, (tt + 1) * 128)
                    o_row = pc.tile([128, HID], F32, tag="orow", bufs=2)
                    for hb in range(NB):
                        o_ps = psc.tile([128, 512], F32, tag="o", bufs=2)
                        for h in range(HPC):
                            nc.tensor.matmul(
                                o_ps,
                                lhsT=attn_n[h][qj][:, tsl],
                                rhs=wo_t[h][:, hb * 512:(hb + 1) * 512],
                                start=(h == 0),
                                stop=(h == HPC - 1),
                            )
                        nc.vector.tensor_copy(o_row[:, hb * 512:(hb + 1) * 512], o_ps)
                    nc.scalar.dma_start(
                        out=out[tb * 128:(tb + 1) * 128, :], in_=o_row)


# ------------------------------ host side ----------------------------------

_NC_CACHE = {}


def _get_nc():
    if "nc" not in _NC_CACHE:
        _NC_CACHE["nc"] = build_bass()
    return _NC_CACHE["nc"]


def make_in_maps(positions, hidden_states, w_q_a, q_a_ln_w, w_q_b, w_kv_a,
                 kv_a_ln_w, w_kv_b, w_o):
    positions = np.asarray(positions)
    hidden_states = np.asarray(hidden_states, dtype=np.float32)
    w_q_a = np.asarray(w_q_a, dtype=np.float32)
    q_a_ln_w = np.asarray(q_a_ln_w, dtype=np.float32)
    w_q_b = np.asarray(w_q_b, dtype=np.float32)
    w_kv_a = np.asarray(w_kv_a, dtype=np.float32)
    kv_a_ln_w = np.asarray(kv_a_ln_w, dtype=np.float32)
    w_kv_b = np.asarray(w_kv_b, dtype=np.float32)
    w_o = np.asarray(w_o, dtype=np.float32)

    hs_t = np.ascontiguousarray(hidden_states.T)

    # de-interleave order for rope dims: evens then odds
    order = np.concatenate([np.arange(0, DR, 2), np.arange(1, DR, 2)])

    wkva_p = w_kv_a.copy()
    wkva_p[:, KVLR:] = w_kv_a[:, KVLR:][:, order]
    wkva_p = np.ascontiguousarray(wkva_p)

    # rope tables (feature-major, de-interleaved: evens;odds)
    inv_freq = 1.0 / (THETA ** (np.arange(0, DR, 2, dtype=np.float64) / DR))
    ang = positions.astype(np.float64)[:, None] * inv_freq[None, :]  # [T, 32]
    cosT = np.cos(ang).T.astype(np.float32)                          # [32, T]
    sinT = np.sin(ang).T.astype(np.float32)
    cosf = np.ascontiguousarray(np.concatenate([cosT, cosT], axis=0))
    sinf = np.ascontiguousarray(np.concatenate([-sinT, sinT], axis=0))

    # block swap permutation (lhsT form; symmetric)
    perm = np.zeros((DR, DR), dtype=np.float32)
    for i in range(DR):
        perm[i, (i + DR // 2) % DR] = 1.0

    # diagonal causal mask patterns: keep if p + 128*sub <= f
    maskd = np.zeros((128, 4 * 512), dtype=np.float32)
    p = np.arange(128)[:, None]
    f = np.arange(512)[None, :]
    for sub in range(4):
        maskd[:, sub * 512:(sub + 1) * 512] = (p + 128 * sub <= f)
    maskd = np.ascontiguousarray(maskd)

    in_maps = []
    for c in range(NCORES):
        h0, h1 = HPC * c, HPC * c + 1
        wqb_c = np.concatenate([
            w_q_b[:, h0 * DQK:h0 * DQK + DN],
            w_q_b[:, h1 * DQK:h1 * DQK + DN],
            w_q_b[:, h0 * DQK + DN:(h0 + 1) * DQK][:, order],
            w_q_b[:, h1 * DQK + DN:(h1 + 1) * DQK][:, order],
        ], axis=1) * q_a_ln_w[:, None] * SCALE
        wkvbk_c = np.concatenate([
            w_kv_b[:, h0 * (DN + DV):h0 * (DN + DV) + DN],
            w_kv_b[:, h1 * (DN + DV):h1 * (DN + DV) + DN],
        ], axis=1) * kv_a_ln_w[:, None]
        wkvbv_c = np.concatenate([
            w_kv_b[:, h0 * (DN + DV) + DN:(h0 + 1) * (DN + DV)],
            w_kv_b[:, h1 * (DN + DV) + DN:(h1 + 1) * (DN + DV)],
        ], axis=1) * kv_a_ln_w[:, None]
        wo_c = np.concatenate([
            w_o[h0 * DV:(h0 + 1) * DV, :],
            w_o[h1 * DV:(h1 + 1) * DV, :],
        ], axis=0)
        tsl = slice(c * TSH, (c + 1) * TSH)
        in_maps.append({
            "hs_sh": np.ascontiguousarray(hs_t[:, tsl]),
            "wqa": w_q_a,
            "wkva": wkva_p,
            "wqb": np.ascontiguousarray(wqb_c.astype(np.float32)),
            "wkvbk": np.ascontiguousarray(wkvbk_c.astype(np.float32)),
            "wkvbv": np.ascontiguousarray(wkvbv_c.astype(np.float32)),
            "wo": np.ascontiguousarray(wo_c.astype(np.float32)),
            "cosf": cosf,
            "sinf": sinf,
            "cosf_sh": np.ascontiguousarray(cosf[:, tsl]),
            "sinf_sh": np.ascontiguousarray(sinf[:, tsl]),
            "perm64": perm,
            "maskd": maskd,
            "ones": np.ones((128, 128), dtype=np.float32),
        })
    return in_maps


def kernel(positions, hidden_states, w_q_a, q_a_ln_w, w_q_b, w_kv_a,
           kv_a_ln_w, w_kv_b, w_o):
    nc = _get_nc()
    in_maps = make_in_maps(positions, hidden_states, w_q_a, q_a_ln_w, w_q_b,
                           w_kv_a, kv_a_ln_w, w_kv_b, w_o)
    res = bass_utils.run_bass_kernel_spmd(nc, in_maps, core_ids=list(range(NCORES)))
    acc = np.zeros((T, HID), dtype=np.float32)
    for c in range(NCORES):
        acc += res.results[c]["out"]
    return acc


# revision 19
# speedup vs baseline: 205.5893x; 191.5096x over previous
"""DeepseekV2 MLA attention on 8 Trainium2 NeuronCores (Bass/Tile).

Tensor-parallel over heads (2 heads/core): w_q_b / w_kv_b output dims and
w_o input dim sharded across cores; q_a / kv_a projections replicated.
Per-core partial outputs are summed on the host (row-parallel unshard).

Self-contained: hardcodes all shapes from the problem spec.
"""

import numpy as np

import concourse.bass as bass
import concourse.bacc as bacc
import concourse.mybir as mybir
import concourse.tile as tile
from concourse import bass_utils

# Problem dims
T = 2048
HID = 2048
H = 16
DN = 128      # qk_nope_head_dim
DR = 64       # qk_rope_head_dim
DV = 128      # v_head_dim
DQK = DN + DR
QLR = 1536    # q_lora_rank
KVLR = 512    # kv_lora_rank
THETA = 10000.0
EPS = 1e-6
SCALE = DQK ** -0.5

NCORES = 8
HPC = H // NCORES            # heads per core = 2
LATR = KVLR + DR             # latent rows = 576

F32 = mybir.dt.float32
F32R = mybir.dt.float32r

KT = HID // 128              # 16 contraction chunks for phase A
QMT = QLR // 128             # 12 q_a row tiles
KVMT = KVLR // 128           # 4 latent (normed) row tiles
NB = T // 512                # 4 token blocks of 512
TBT = T // 128               # 16 token tiles of 128
TSH = T // NCORES            # 256 tokens per core shard





def build_bass():
    nc = bacc.Bacc(
        "TRN2",
        target_bir_lowering=False,
        debug=False,
        enable_asserts=False,
        num_devices=NCORES,
    )

    hs_sh = nc.dram_tensor("hs_sh", [HID, TSH], F32R, kind="ExternalInput").ap()
    wqa = nc.dram_tensor("wqa", [HID, QLR], F32R, kind="ExternalInput").ap()
    wkva = nc.dram_tensor("wkva", [HID, LATR], F32R, kind="ExternalInput").ap()
    wqb = nc.dram_tensor("wqb", [QLR, HPC * DQK], F32R, kind="ExternalInput").ap()
    wkvbk = nc.dram_tensor("wkvbk", [KVLR, HPC * DN], F32R, kind="ExternalInput").ap()
    wkvbv = nc.dram_tensor("wkvbv", [KVLR, HPC * DV], F32R, kind="ExternalInput").ap()
    wo = nc.dram_tensor("wo", [HPC * DV, HID], F32R, kind="ExternalInput").ap()
    cosf = nc.dram_tensor("cosf", [DR, T], F32R, kind="ExternalInput").ap()
    sinf = nc.dram_tensor("sinf", [DR, T], F32R, kind="ExternalInput").ap()
    cosf_sh = nc.dram_tensor("cosf_sh", [DR, TSH], F32R, kind="ExternalInput").ap()
    sinf_sh = nc.dram_tensor("sinf_sh", [DR, TSH], F32R, kind="ExternalInput").ap()
    perm64 = nc.dram_tensor("perm64", [DR, DR], F32R, kind="ExternalInput").ap()
    maskd = nc.dram_tensor("maskd", [128, 4 * 512], F32R, kind="ExternalInput").ap()
    ones = nc.dram_tensor("ones", [128, 128], F32R, kind="ExternalInput").ap()
    out = nc.dram_tensor("out", [T, HID], F32, kind="ExternalOutput").ap()

    with tile.TileContext(nc) as tc:
        _kernel_body(nc, tc, hs_sh, wqa, wkva, wqb, wkvbk, wkvbv, wo,
                     cosf, sinf, cosf_sh, sinf_sh, perm64, maskd, ones, out)

    nc.compile()
    return nc


def _kernel_body(nc, tc, hs_sh, wqa, wkva, wqb, wkvbk, wkvbv, wo,
                 cosf, sinf, cosf_sh, sinf_sh, perm64, maskd, ones, out):
    from contextlib import ExitStack

    ctx = ExitStack()
    with ctx:
        dram = ctx.enter_context(tc.tile_pool(name="dram", bufs=1, space="DRAM"))
        contrib_q = dram.tile([QLR, TSH], F32R)
        contrib_kv = dram.tile([LATR, TSH], F32R)
        gath_q = dram.tile([NCORES * QLR, TSH], F32R, addr_space="Shared")
        gath_kv = dram.tile([NCORES * LATR, TSH], F32R, addr_space="Shared")

        persist = ctx.enter_context(tc.tile_pool(name="persist", bufs=1))

        ones128 = persist.tile([128, 128], F32R, tag="ones128")
        nc.sync.dma_start(out=ones128, in_=ones)
        ones_col = ones128[:, 0:1]
        ones_row = ones128[0:1, :]
        perm_t0 = persist.tile([DR, DR], F32R, tag="perm0")
        nc.sync.dma_start(out=perm_t0, in_=perm64)
        cosf_sh_t = persist.tile([DR, TSH], F32R, tag="cosfsh")
        nc.sync.dma_start(out=cosf_sh_t, in_=cosf_sh)
        sinf_sh_t = persist.tile([DR, TSH], F32R, tag="sinfsh")
        nc.sync.dma_start(out=sinf_sh_t, in_=sinf_sh)


        # ------- Phase A: token-sharded q_a^T / latent^T, norm + rope local ----
        with tc.tile_pool(name="pa", bufs=1) as pa, \
             tc.tile_pool(name="psa", bufs=1, space="PSUM") as psa:
            hst = []
            for k in range(KT):
                h = pa.tile([128, TSH], F32R, tag=f"hs{k}")
                nc.sync.dma_start(out=h, in_=hs_sh[k * 128:(k + 1) * 128, :])
                hst.append(h)

            def a_mtile(w_dram, m_cols, mrows, z_tile, z_start, z_stop, stg_tag):
                wstrip = pa.tile([128, KT, mrows], F32R, tag="wstrip", bufs=2)
                nc.scalar.dma_start(
                    out=wstrip,
                    in_=w_dram[:, m_cols[0]:m_cols[1]].rearrange(
                        "(kc p) m -> p kc m", p=128),
                )
                pq = psa.tile([mrows, TSH], F32, tag="pq", bufs=3)
                for k in range(KT):
                    nc.tensor.matmul(
                        pq, lhsT=wstrip[:, k, :], rhs=hst[k],
                        start=(k == 0), stop=(k == KT - 1))
                stage = pa.tile([mrows, TSH], F32R, tag=stg_tag, name=stg_tag)
                nc.vector.tensor_copy(stage, pq)
                if z_tile is not None:
                    sq = pa.tile([mrows, TSH], F32R, tag="sq", bufs=2)
                    nc.scalar.square(sq, stage)
                    nc.tensor.matmul(z_tile, lhsT=ones_col[0:mrows, :], rhs=sq,
                                     start=z_start, stop=z_stop)
                return stage

            def rsqrt_bc(z_psum, n, tag):
                # [128, TSH] broadcast of 1/sqrt(z/n + eps)
                tmp = pa.tile([1, TSH], F32, tag="rsq_tmp", bufs=2)
                nc.scalar.activation(tmp, z_psum,
                                     mybir.ActivationFunctionType.Copy,
                                     bias=EPS, scale=1.0 / n)
                nc.vector.reciprocal(tmp, tmp)
                srow = pa.tile([1, TSH], F32R, tag=tag + "r", name=tag + "r")
                nc.scalar.activation(srow, tmp,
                                     mybir.ActivationFunctionType.Sqrt)
                b_ps = psa.tile([128, TSH], F32, tag="bc", bufs=2)
                nc.tensor.matmul(b_ps, lhsT=ones_row, rhs=srow,
                                 start=True, stop=True)
                bc = pa.tile([128, TSH], F32R, tag=tag, name=tag)
                nc.scalar.copy(bc, b_ps)
                return bc

            zq = psa.tile([1, TSH], F32, tag="z")
            q_stages = []
            for m in range(QMT):
                q_stages.append(a_mtile(wqa, (m * 128, (m + 1) * 128), 128,
                                        zq, m == 0, m == QMT - 1, f"stq{m}"))
            sq_bc = rsqrt_bc(zq, QLR, "sqbc")

            zkv = psa.tile([1, TSH], F32, tag="z")
            kv_stages = []
            for m in range(KVMT):
                kv_stages.append(a_mtile(wkva, (m * 128, (m + 1) * 128), 128,
                                         zkv, m == 0, m == KVMT - 1, f"stkv{m}"))
            skv_bc = rsqrt_bc(zkv, KVLR, "skvbc")

            kpe_stage = a_mtile(wkva, (KVLR, LATR), DR, None, False, False,
                                "stkpe")

            # normalize + write contributions
            for m in range(QMT):
                qs = pa.tile([128, TSH], F32R, tag="qnorm", bufs=3,
                             name=f"qnorm{m}")
                nc.vector.tensor_tensor(qs, q_stages[m], sq_bc,
                                        op=mybir.AluOpType.mult)
                nc.sync.dma_start(
                    out=contrib_q[m * 128:(m + 1) * 128, :], in_=qs)
            for m in range(KVMT):
                ks = pa.tile([128, TSH], F32R, tag="kvnorm", bufs=2,
                             name=f"kvnorm{m}")
                nc.vector.tensor_tensor(ks, kv_stages[m], skv_bc,
                                        op=mybir.AluOpType.mult)
                nc.sync.dma_start(
                    out=contrib_kv[m * 128:(m + 1) * 128, :], in_=ks)
            # rope k_pe locally
            sw_ps = psa.tile([DR, TSH], F32, tag="bc", bufs=2)
            nc.tensor.matmul(sw_ps, lhsT=perm_t0, rhs=kpe_stage,
                             start=True, stop=True)
            rt1 = pa.tile([DR, TSH], F32R, tag="rt1")
            nc.vector.tensor_tensor(rt1, kpe_stage, cosf_sh_t,
                                    op=mybir.AluOpType.mult)
            rt2 = pa.tile([DR, TSH], F32R, tag="rt2")
            nc.vector.tensor_tensor(rt2, sw_ps, sinf_sh_t,
                                    op=mybir.AluOpType.mult)
            kpe_roped = pa.tile([DR, TSH], F32R, tag="kper")
            nc.vector.tensor_tensor(kpe_roped, rt1, rt2,
                                    op=mybir.AluOpType.add)
            nc.sync.dma_start(out=contrib_kv[KVLR:LATR, :], in_=kpe_roped)

            # all-gather (kv first so B's kv-side work unblocks early)
            nc.gpsimd.collective_compute(
                "AllGather", mybir.AluOpType.bypass,
                replica_groups=[list(range(NCORES))],
                ins=[contrib_kv], outs=[gath_kv])
            nc.gpsimd.collective_compute(
                "AllGather", mybir.AluOpType.bypass,
                replica_groups=[list(range(NCORES))],
                ins=[contrib_q], outs=[gath_q])

        # ---------------- Phase B: per-head q/k/v + rope ------------------------
        qn = [[None] * NB for _ in range(HPC)]     # [128, 512] nope q, f-major
        qpe = [[None] * NB for _ in range(HPC)]    # [64, 512] roped q pe
        kn = [[None] * NB for _ in range(HPC)]     # [128, 512] k nope, f-major
        kpe = [None] * NB                          # [64, 512] roped k pe (shared)
        vt = [None] * TBT                          # [128, 256] v token-major, 2 heads

        bcp = ctx.enter_context(tc.tile_pool(name="bcp", bufs=1))

        with tc.tile_pool(name="pb", bufs=1) as pb, \
             tc.tile_pool(name="psb", bufs=1, space="PSUM") as psb:
            # resident weights
            wqb_t = pb.tile([128, QMT, HPC * DQK], F32R, tag="wqb")
            nc.sync.dma_start(
                out=wqb_t, in_=wqb.rearrange("(kc p) m -> p kc m", p=128))
            wkvbk_t = pb.tile([128, KVMT, HPC * DN], F32R, tag="wkvbk")
            nc.sync.dma_start(
                out=wkvbk_t, in_=wkvbk.rearrange("(kc p) m -> p kc m", p=128))
            wkvbv_t = pb.tile([128, KVMT, HPC * DV], F32R, tag="wkvbv")
            nc.sync.dma_start(
                out=wkvbv_t, in_=wkvbv.rearrange("(kc p) m -> p kc m", p=128))
            cosf_t = pb.tile([DR, T], F32R, tag="cosf")
            nc.sync.dma_start(out=cosf_t, in_=cosf)
            sinf_t = pb.tile([DR, T], F32R, tag="sinf")
            nc.sync.dma_start(out=sinf_t, in_=sinf)
            perm_t = pb.tile([DR, DR], F32R, tag="perm")
            nc.sync.dma_start(out=perm_t, in_=perm64)

            def rope(dst, raw, blk):
                """raw [64, 512] (evens;odds) -> roped dst [64, 512]."""
                sl = slice(blk * 512, (blk + 1) * 512)
                sw_ps = psb.tile([DR, 512], F32, tag="swp", bufs=2)
                nc.tensor.matmul(sw_ps, lhsT=perm_t, rhs=raw,
                                 start=True, stop=True)
                t1 = pb.tile([DR, 512], F32R, tag="ropet1", bufs=2)
                nc.vector.tensor_tensor(t1, raw, cosf_t[:, sl],
                                        op=mybir.AluOpType.mult)
                t2 = pb.tile([DR, 512], F32R, tag="ropet2", bufs=2)
                nc.vector.tensor_tensor(t2, sw_ps, sinf_t[:, sl],
                                        op=mybir.AluOpType.mult)
                nc.vector.tensor_tensor(dst, t1, t2, op=mybir.AluOpType.add)

            for j in range(NB):
                jsl = slice(j * 512, (j + 1) * 512)
                # ---- q_b matmuls: 4 out tiles (h0n, h1n, h0pe, h1pe) ----
                accs = []
                for mt, rows in ((0, 128), (1, 128), (2, DR), (3, DR)):
                    accs.append(psb.tile([rows, 512], F32, tag="acc", bufs=4, name=f"accq{mt}"))
                col_of = (0, DN, 2 * DN, 2 * DN + DR)
                rows_of = (128, 128, DR, DR)
                for k in range(QMT):
                    qa_ch = pb.tile([128, 512], F32R, tag="qa", bufs=4)
                    eng = nc.scalar if k % 2 == 0 else nc.sync
                    for half in range(2):
                        base = (2 * j + half) * QLR + k * 128
                        eng.dma_start(
                            out=qa_ch[:, half * TSH:(half + 1) * TSH],
                            in_=gath_q[base:base + 128, :])
                    for mt in range(4):
                        nc.tensor.matmul(
                            accs[mt],
                            lhsT=wqb_t[:, k, col_of[mt]:col_of[mt] + rows_of[mt]],
                            rhs=qa_ch,
                            start=(k == 0),
                            stop=(k == QMT - 1),
                        )
                for h in range(HPC):
                    qn[h][j] = bcp.tile([128, 512], F32R, tag=f"qn{h}_{j}", name=f"qn{h}_{j}")
                    nc.vector.tensor_copy(qn[h][j], accs[h])
                for h in range(HPC):
                    qpe_raw = pb.tile([DR, 512], F32R, tag="qperaw", bufs=2)
                    nc.vector.tensor_copy(qpe_raw, accs[2 + h])
                    qpe[h][j] = bcp.tile([DR, 512], F32R, tag=f"qpe{h}_{j}", name=f"qpe{h}_{j}")
                    rope(qpe[h][j], qpe_raw, j)

                # ---- latent chunks (pre-normalized) + roped k_pe from gather ----
                kva_n = []
                for k in range(KVMT):
                    kvn = pb.tile([128, 512], F32R, tag="kvan", bufs=4)
                    for half in range(2):
                        base = (2 * j + half) * LATR + k * 128
                        nc.sync.dma_start(
                            out=kvn[:, half * TSH:(half + 1) * TSH],
                            in_=gath_kv[base:base + 128, :])
                    kva_n.append(kvn)
                kpe[j] = bcp.tile([DR, 512], F32R, tag=f"kpe_{j}", name=f"kpe_{j}")
                for half in range(2):
                    base = (2 * j + half) * LATR + KVLR
                    nc.sync.dma_start(
                        out=kpe[j][:, half * TSH:(half + 1) * TSH],
                        in_=gath_kv[base:base + DR, :])

                # ---- k_nope ----
                for h in range(HPC):
                    acck = psb.tile([128, 512], F32, tag="acc", bufs=4)
                    for k in range(KVMT):
                        nc.tensor.matmul(
                            acck,
                            lhsT=wkvbk_t[:, k, h * DN:(h + 1) * DN],
                            rhs=kva_n[k],
                            start=(k == 0),
                            stop=(k == KVMT - 1),
                        )
                    kn[h][j] = bcp.tile([128, 512], F32R, tag=f"kn{h}_{j}", name=f"kn{h}_{j}")
                    nc.vector.tensor_copy(kn[h][j], acck)

                # ---- v (token-major, both heads packed) ----
                for tt in range(4):
                    tb = j * 4 + tt
                    accv = psb.tile([128, HPC * DV], F32, tag="acc", bufs=4)
                    for k in range(KVMT):
                        nc.tensor.matmul(
                            accv,
                            lhsT=kva_n[k][:, tt * 128:(tt + 1) * 128],
                            rhs=wkvbv_t[:, k, :],
                            start=(k == 0),
                            stop=(k == KVMT - 1),
                        )
                    vt[tb] = bcp.tile([128, HPC * DV], F32R, tag=f"v_{tb}", name=f"v_{tb}")
                    nc.vector.tensor_copy(vt[tb], accv)

        # ---------------- Attention + output projection -------------------------
        with tc.tile_pool(name="pc", bufs=1) as pc, \
             tc.tile_pool(name="psc", bufs=1, space="PSUM") as psc:
            maskd_t = pc.tile([128, 4 * 512], F32R, tag="maskd")
            nc.sync.dma_start(out=maskd_t, in_=maskd)
            wo_t = []
            for h in range(HPC):
                w = pc.tile([128, HID], F32R, tag=f"wo{h}")
                nc.sync.dma_start(out=w, in_=wo[h * DV:(h + 1) * DV, :])
                wo_t.append(w)

            attn_n = [[None] * NB for _ in range(HPC)]
            for qj in range(NB):
                nki = 4 * qj + 4
                for h in range(HPC):
                    attn_ps = psc.tile([128, 512], F32, tag="attn", bufs=2)
                    z_ps = psc.tile([1, 512], F32, tag="zr", bufs=1)
                    for ki in range(nki):
                        jb, sub = ki // 4, ki % 4
                        ksl = slice(sub * 128, (sub + 1) * 128)
                        s_ps = psc.tile([128, 512], F32, tag="s", bufs=3)
                        nc.tensor.matmul(s_ps, lhsT=kn[h][jb][:, ksl],
                                         rhs=qn[h][qj],
                                         start=True, stop=False)
                        nc.tensor.matmul(s_ps, lhsT=kpe[jb][:, ksl],
                                         rhs=qpe[h][qj],
                                         start=False, stop=True)
                        e = pc.tile([128, 512], F32R, tag="e", bufs=4)
                        nc.scalar.activation(e, s_ps,
                                             mybir.ActivationFunctionType.Exp)
                        if ki >= 4 * qj:  # diagonal block: causal mask
                            sub_d = ki - 4 * qj
                            nc.vector.tensor_tensor(
                                e, e, maskd_t[:, sub_d * 512:(sub_d + 1) * 512],
                                op=mybir.AluOpType.mult)
                        nc.tensor.matmul(z_ps, lhsT=ones_col, rhs=e,
                                         start=(ki == 0), stop=(ki == nki - 1))
                        nc.tensor.matmul(attn_ps,
                                         lhsT=vt[ki][:, h * DV:(h + 1) * DV],
                                         rhs=e,
                                         start=(ki == 0), stop=(ki == nki - 1))
                    rz = pc.tile([1, 512], F32R, tag="rz", bufs=2)
                    with nc.allow_low_precision(reason="fp32r softmax denom"):
                        nc.vector.reciprocal(rz, z_ps)
                    bc_ps = psc.tile([128, 512], F32, tag="s", bufs=3)
                    nc.tensor.matmul(bc_ps, lhsT=ones_row, rhs=rz,
                                     start=True, stop=True)
                    bc_sb = pc.tile([128, 512], F32R, tag="bcs", bufs=2)
                    nc.scalar.copy(bc_sb, bc_ps)
                    attn_n[h][qj] = bcp.tile([128, 512], F32R, tag=f"attn{h}_{qj}", name=f"attn{h}_{qj}")
                    nc.vector.tensor_tensor(attn_n[h][qj], attn_ps, bc_sb,
                                            op=mybir.AluOpType.mult)

                # output projection for this qj's 4 token tiles
                for tt in range(4):
                    tb = qj * 4 + tt
                    tsl = slice(tt * 128, (tt + 1) * 128)
                    o_row = pc.tile([128, HID], F32, tag="orow", bufs=2)
                    for hb in range(NB):
                        o_ps = psc.tile([128, 512], F32, tag="o", bufs=2)
                        for h in range(HPC):
                            nc.tensor.matmul(
                                o_ps,
                                lhsT=attn_n[h][qj][:, tsl],
                                rhs=wo_t[h][:, hb * 512:(hb + 1) * 512],
                                start=(h == 0),
                                stop=(h == HPC - 1),
                            )
                        nc.vector.tensor_copy(o_row[:, hb * 512:(hb + 1) * 512], o_ps)
                    nc.scalar.dma_start(
                        out=out[tb * 128:(tb + 1) * 128, :], in_=o_row)


# ------------------------------ host side ----------------------------------

_NC_CACHE = {}


def _get_nc():
    if "nc" not in _NC_CACHE:
        _NC_CACHE["nc"] = build_bass()
    return _NC_CACHE["nc"]


def make_in_maps(positions, hidden_states, w_q_a, q_a_ln_w, w_q_b, w_kv_a,
                 kv_a_ln_w, w_kv_b, w_o):
    positions = np.asarray(positions)
    hidden_states = np.asarray(hidden_states, dtype=np.float32)
    w_q_a = np.asarray(w_q_a, dtype=np.float32)
    q_a_ln_w = np.asarray(q_a_ln_w, dtype=np.float32)
    w_q_b = np.asarray(w_q_b, dtype=np.float32)
    w_kv_a = np.asarray(w_kv_a, dtype=np.float32)
    kv_a_ln_w = np.asarray(kv_a_ln_w, dtype=np.float32)
    w_kv_b = np.asarray(w_kv_b, dtype=np.float32)
    w_o = np.asarray(w_o, dtype=np.float32)

    hs_t = np.ascontiguousarray(hidden_states.T)

    # de-interleave order for rope dims: evens then odds
    order = np.concatenate([np.arange(0, DR, 2), np.arange(1, DR, 2)])

    wkva_p = w_kv_a.copy()
    wkva_p[:, KVLR:] = w_kv_a[:, KVLR:][:, order]
    wkva_p = np.ascontiguousarray(wkva_p)

    # rope tables (feature-major, de-interleaved: evens;odds)
    inv_freq = 1.0 / (THETA ** (np.arange(0, DR, 2, dtype=np.float64) / DR))
    ang = positions.astype(np.float64)[:, None] * inv_freq[None, :]  # [T, 32]
    cosT = np.cos(ang).T.astype(np.float32)                          # [32, T]
    sinT = np.sin(ang).T.astype(np.float32)
    cosf = np.ascontiguousarray(np.concatenate([cosT, cosT], axis=0))
    sinf = np.ascontiguousarray(np.concatenate([-sinT, sinT], axis=0))

    # block swap permutation (lhsT form; symmetric)
    perm = np.zeros((DR, DR), dtype=np.float32)
    for i in range(DR):
        perm[i, (i + DR // 2) % DR] = 1.0

    # diagonal causal mask patterns: keep if p + 128*sub <= f
    maskd = np.zeros((128, 4 * 512), dtype=np.float32)
    p = np.arange(128)[:, None]
    f = np.arange(512)[None, :]
    for sub in range(4):
        maskd[:, sub * 512:(sub + 1) * 512] = (p + 128 * sub <= f)
    maskd = np.ascontiguousarray(maskd)

    in_maps = []
    for c in range(NCORES):
        h0, h1 = HPC * c, HPC * c + 1
        wqb_c = np.concatenate([
            w_q_b[:, h0 * DQK:h0 * DQK + DN],
            w_q_b[:, h1 * DQK:h1 * DQK + DN],
            w_q_b[:, h0 * DQK + DN:(h0 + 1) * DQK][:, order],
            w_q_b[:, h1 * DQK + DN:(h1 + 1) * DQK][:, order],
        ], axis=1) * q_a_ln_w[:, None] * SCALE
        wkvbk_c = np.concatenate([
            w_kv_b[:, h0 * (DN + DV):h0 * (DN + DV) + DN],
            w_kv_b[:, h1 * (DN + DV):h1 * (DN + DV) + DN],
        ], axis=1) * kv_a_ln_w[:, None]
        wkvbv_c = np.concatenate([
            w_kv_b[:, h0 * (DN + DV) + DN:(h0 + 1) * (DN + DV)],
            w_kv_b[:, h1 * (DN + DV) + DN:(h1 + 1) * (DN + DV)],
        ], axis=1) * kv_a_ln_w[:, None]
        wo_c = np.concatenate([
            w_o[h0 * DV:(h0 + 1) * DV, :],
            w_o[h1 * DV:(h1 + 1) * DV, :],
        ], axis=0)
        tsl = slice(c * TSH, (c + 1) * TSH)
        in_maps.append({
            "hs_sh": np.ascontiguousarray(hs_t[:, tsl]),
            "wqa": w_q_a,
            "wkva": wkva_p,
            "wqb": np.ascontiguousarray(wqb_c.astype(np.float32)),
            "wkvbk": np.ascontiguousarray(wkvbk_c.astype(np.float32)),
            "wkvbv": np.ascontiguousarray(wkvbv_c.astype(np.float32)),
            "wo": np.ascontiguousarray(wo_c.astype(np.float32)),
            "cosf": cosf,
            "sinf": sinf,
            "cosf_sh": np.ascontiguousarray(cosf[:, tsl]),
            "sinf_sh": np.ascontiguousarray(sinf[:, tsl]),
            "perm64": perm,
            "maskd": maskd,
            "ones": np.ones((128, 128), dtype=np.float32),
        })
    return in_maps


def kernel(positions, hidden_states, w_q_a, q_a_ln_w, w_q_b, w_kv_a,
           kv_a_ln_w, w_kv_b, w_o):
    nc = _get_nc()
    in_maps = make_in_maps(positions, hidden_states, w_q_a, q_a_ln_w, w_q_b,
                           w_kv_a, kv_a_ln_w, w_kv_b, w_o)
    res = bass_utils.run_bass_kernel_spmd(nc, in_maps, core_ids=list(range(NCORES)))
    acc = np.zeros((T, HID), dtype=np.float32)
    for c in range(NCORES):
        acc += res.results[c]["out"]
    return acc
